# revision 10
# baseline (speedup 1.0000x reference)
"""Trainium2 Bass kernel for nn_BandSplit (grouped band einsum as banded matmul).

The reference computes, per (b, t) row:
    g = gather(x, f_idxes) * mask            # per-band slice of the spectrum
    h = einsum('ki,kio->ko', g, pre_weight) + pre_bias
    y = einsum('ko,koj->kj', h, post_weight) + post_bias
    out = scatter_add(y * mask) / ola_window

Because each band's nonzero bins are a contiguous f-range, the whole pipeline
is linear in x and collapses to ONE banded matrix multiply in the interleaved
linear space  lin = f*4 + c  (bandwidth <= 131 < 132):

    out_lin[l', r] = sum_l A[l, l'] * x_lin[l, r]
    A = sum_k scatter(diag(mask_k) @ W1_k @ W2_k @ diag(mask_k / ola))

A is built on the host from the (small) weight inputs.  x is pre-transposed on
the host into [lin, rows] tiles so the device does only contiguous DMA plus
dense 128x128x512 matmuls on 3 block-diagonals (verified: no band couples
tiles further than +-1 apart).  Output tiles are disjoint across cores
(band-parallel sharding over the linear axis).  The bias contribution is a
per-(c, f) constant (independent of b, t), added on the host.
"""

import numpy as np

# ---- problem constants (hardcoded; harness supplies matching inputs) ----
B, C, T, F = 4, 4, 512, 1025
KB, WMAX = 256, 33
L = F * C                 # 4100 linear positions
NT = (L + 127) // 128     # 33 tiles of 128
LPAD = NT * 128           # 4224
R = B * T                 # 2048 rows (b, t)
NCORES = 8
NOUT = 5                  # max out-tiles per core (uniform SPMD program)
NIN = NOUT + 2            # with +-1 halo
CHUNK = 512
NCHUNK = R // CHUNK
ND = 3                    # block diagonals

# out-tile assignment: contiguous runs, [5,4,4,4,4,4,4,4]
_TPC = [NT // NCORES + (1 if i < NT % NCORES else 0) for i in range(NCORES)]
_J0 = [sum(_TPC[:i]) for i in range(NCORES)]  # first global out-tile per core

# dtype plan: x shipped fp16 (SWDGE DMA upcasts to fp32r in-flight, exact:
# 10-bit mantissa fits fp32r's 11), weights fp32r, PSUM fp32, output fp16.
X_DT = "f16"     # "f32r" | "f16"
OUT_DT = "f16"   # "f32"  | "f16"

_prog_cache = {}


def _build_program(loop_iters=1):
    """Uniform SPMD program: per core, NOUT out-tiles x 3 diagonals of
    [128,128] fp32r matmuls over [128,512] row chunks."""
    import concourse.bacc as bacc
    import concourse.tile as tile
    import concourse.mybir as mybir

    key = loop_iters
    if key in _prog_cache:
        return _prog_cache[key]

    f32 = mybir.dt.float32
    f32r = mybir.dt.float32r
    f16 = mybir.dt.float16

    x_dram_dt = f16 if X_DT == "f16" else f32r
    out_dt = f16 if OUT_DT == "f16" else f32

    nc = bacc.Bacc("TRN2", target_bir_lowering=False, debug=False,
                   num_devices=NCORES)
    xin = nc.dram_tensor("xin", [NIN * 128, R], x_dram_dt,
                         kind="ExternalInput").ap()
    wts = nc.dram_tensor("wts", [128, NOUT * ND * 128], f32r,
                         kind="ExternalInput").ap()
    out = nc.dram_tensor("out", [NOUT * 128, R], out_dt,
                         kind="ExternalOutput").ap()

    with tile.TileContext(nc) as tc:
        with (
            tc.tile_pool(name="xp", bufs=1) as xp,
            tc.tile_pool(name="wp", bufs=1) as wp,
            tc.tile_pool(name="yp", bufs=3) as yp,
            tc.tile_pool(name="pp", bufs=8, space="PSUM") as pp,
        ):
            def body(_iv=None):
                xs = []
                for i in range(NIN):
                    t = xp.tile([128, R], f32r, tag=f"x{i}")
                    if X_DT == "f16":
                        # SWDGE casts fp16 -> fp32r in-flight (exact)
                        nc.gpsimd.dma_start(t[:],
                                            xin[i * 128:(i + 1) * 128, :])
                    else:
                        nc.sync.dma_start(t[:],
                                          xin[i * 128:(i + 1) * 128, :])
                    xs.append(t)
                wt = wp.tile([128, NOUT * ND * 128], f32r, tag="w")
                nc.sync.dma_start(wt[:], wts)
                for j in range(NOUT):
                    y = yp.tile([128, R], out_dt, tag="y")
                    for ch in range(NCHUNK):
                        ps = pp.tile([128, CHUNK], f32, tag="ps")
                        for d in range(ND):
                            blk = (j * ND + d) * 128
                            nc.tensor.matmul(
                                ps[:],
                                wt[:, blk:blk + 128],
                                xs[j + d][:, ch * CHUNK:(ch + 1) * CHUNK],
                                start=(d == 0), stop=(d == ND - 1),
                            )
                        dst = y[:, ch * CHUNK:(ch + 1) * CHUNK]
                        if ch % 2 == 0:
                            nc.scalar.copy(dst, ps[:])
                        else:
                            nc.vector.tensor_copy(dst, ps[:])
                    nc.sync.dma_start(out[j * 128:(j + 1) * 128, :], y[:])

            if loop_iters == 1:
                body()
            else:
                with tc.For_i(0, loop_iters, 1) as _i:
                    body(_i)

    nc.compile()
    _prog_cache[key] = nc
    return nc


def _build_A(pre_weight, pre_bias, post_weight, post_bias, mask, ola_window,
             f_idxes):
    """Host: banded operator A[in_lin, out_lin] (LPAD x LPAD, fp32) and the
    constant bias image (C, F)."""
    fi = f_idxes.reshape(KB, WMAX).astype(np.int64)
    mk = mask.reshape(KB, WMAX).astype(np.float32)
    ola = ola_window.astype(np.float32)

    # effective per-band operators with mask and 1/ola folded in
    # row (input) index i = w*C + c ; col (output) index j = w'*C + c'
    mrow = np.repeat(mk, C, axis=1)                     # (KB, WMAX*C)
    inv_ola = np.where(ola != 0, 1.0 / ola, 0.0)
    ola_cols = inv_ola[fi]                              # (KB, WMAX)
    mcol = np.repeat(mk * ola_cols, C, axis=1)          # (KB, WMAX*C)

    w1 = pre_weight * mrow[:, :, None]                  # (KB, D, 128)
    w2 = post_weight * mcol[:, None, :]                 # (KB, 128, D)
    Mk = np.matmul(w1, w2)                              # (KB, D, D) fp32

    A = np.zeros((LPAD, LPAD), np.float32)
    lin = (fi[:, :, None] * C + np.arange(C)[None, None, :]).reshape(KB, -1)
    for k in range(KB):
        idx = lin[k]
        A[np.ix_(idx, idx)] += Mk[k]   # duplicate idx entries are all-zero rows/cols

    # bias: (pre_bias @ W2_raw + post_bias) * mask / ola, scattered -> (C, F)
    by = (np.einsum('ko,koj->kj', pre_bias, post_weight) + post_bias)  # (KB, D)
    by = by * mcol                                                      # masked + /ola
    bias_img = np.zeros((C, F), np.float32)
    np.add.at(bias_img,
              (np.tile(np.arange(C), (KB, WMAX, 1)).reshape(KB, -1),
               np.repeat(fi, C, axis=1)),
              by)
    return A, bias_img


def _round_fp32r(a):
    """Round fp32 to the fp32r format (11-bit mantissa, low 12 bits zero),
    round-to-nearest-even.  The PE reads only the top 20 bits; pre-rounding
    on the host keeps RNE accuracy instead of HW truncation."""
    b = np.ascontiguousarray(a, np.float32).view(np.uint32)
    r = (b + 0x7FF + ((b >> 12) & 1)) & np.uint32(0xFFFFF000)
    return r.view(np.float32)


def _shard_inputs(x, A):
    """Per-core xin ([NIN*128, R]) and wts ([128, NOUT*ND*128]) arrays."""
    # x (B, C, T, F) -> X_lin [L, R], lin = f*4+c, r = b*T+t
    X = np.ascontiguousarray(
        x.transpose(3, 1, 0, 2).reshape(L, R).astype(np.float32))
    # rows: 128 front halo + LPAD + 256 tail (halo + core-7's padded 5th tile)
    Xp = np.zeros((LPAD + 384, R), np.float32)
    Xp[128:128 + L] = X                                   # halo offset 128
    Ap = np.zeros((LPAD + 256, LPAD), np.float32)
    Ap[128:128 + LPAD] = A

    in_maps = []
    for cid in range(NCORES):
        j0 = _J0[cid]
        ntile = _TPC[cid]
        xsl = Xp[j0 * 128:(j0 + NIN) * 128]
        if X_DT == "f16":
            xin = xsl.astype(np.float16)
        else:
            xin = _round_fp32r(xsl)
        wts = np.zeros((128, NOUT * ND * 128), np.float32)
        for j in range(ntile):
            gj = j0 + j
            for d in range(ND):
                blk = (j * ND + d) * 128
                wts[:, blk:blk + 128] = Ap[(gj + d) * 128:(gj + d + 1) * 128,
                                           gj * 128:(gj + 1) * 128]
        in_maps.append({"xin": xin, "wts": _round_fp32r(wts)})
    return in_maps


def _gather_output(results, bias_img):
    out_lin = np.zeros((LPAD, R), np.float32)
    for cid in range(NCORES):
        j0, ntile = _J0[cid], _TPC[cid]
        out_lin[j0 * 128:(j0 + ntile) * 128] = \
            results[cid]["out"][:ntile * 128].astype(np.float32)
    # [L, R] -> (B, C, T, F):  lin = f*4+c, r = b*T+t
    out = out_lin[:L].reshape(F, C, B, T).transpose(2, 1, 3, 0)
    out = np.ascontiguousarray(out) + bias_img[None, :, None, :]
    return out.astype(np.float32)


def _run_on_device(in_maps, loop_iters=1):
    from concourse.bass_utils import run_bass_kernel_spmd
    nc = _build_program(loop_iters)
    res = run_bass_kernel_spmd(nc, in_maps, list(range(NCORES)))
    return res.results


def kernel(x, pre_weight, pre_bias, post_weight, post_bias, mask, ola_window,
           f_idxes):
    x = np.asarray(x, np.float32)
    pre_weight = np.asarray(pre_weight, np.float32)
    pre_bias = np.asarray(pre_bias, np.float32)
    post_weight = np.asarray(post_weight, np.float32)
    post_bias = np.asarray(post_bias, np.float32)
    mask = np.asarray(mask, np.float32)
    ola_window = np.asarray(ola_window, np.float32)
    f_idxes = np.asarray(f_idxes)

    A, bias_img = _build_A(pre_weight, pre_bias, post_weight, post_bias,
                           mask, ola_window, f_idxes)
    in_maps = _shard_inputs(x, A)
    results = _run_on_device(in_maps)
    return _gather_output(results, bias_img)
